# revision 10
# baseline (speedup 1.0000x reference)
"""GroupedQueryAttention Trainium2 Bass kernel (v2, bf16).

Sharding: 8 cores = 2 (batch) x 4 (KV groups). Each core computes, for its
(b, g): q/k/v projections for the group's 4 query heads + 1 kv head, causal
attention, and the partial output projection ctx_g @ Wo[g-rows]. Host sums
the 4 group partials per batch and adds the bias.

v2 changes vs v1:
- All HBM tensors bf16, host-packed into [128, N] layouts so every DMA is a
  full-partition contiguous transfer (26 DMAs total vs 136; ~1/4 the
  descriptors, half the bytes).
- Attention loops ki-outer / head-inner so 4 independent S->exp->PV chains
  keep PE busy while ACT computes exp.
- Softmax row-sums for 2 heads packed per PSUM bank at partition offsets
  {0, 64}: ctxp 4 banks + st 2 + rs 2 = 8 banks exactly.
- All matmuls bf16 (1 cycle/row at any width) with fp32 PSUM accumulation.
"""
import sys
sys.path.insert(0, '/opt/trn_rl_repo')

import numpy as np
import ml_dtypes
import concourse.bass as bass
import concourse.bacc as bacc
import concourse.tile as tile
import concourse.mybir as mybir
from concourse import bass_utils
from concourse.bass_interp import get_hw_module
from contextlib import ExitStack, nullcontext

F32 = mybir.dt.float32
F32R = mybir.dt.float32r
BF16 = mybir.dt.bfloat16
AF = mybir.ActivationFunctionType
ALU = mybir.AluOpType
BF = ml_dtypes.bfloat16

SEQ = 2048
D = 2048
HD = 128          # head dim
NHL = 4           # query heads per core (group size)
QC = 512          # query chunk
NQC = SEQ // QC   # 4
NKT = SEQ // 128  # 16 key tiles
NDT = D // 128    # 16 contraction tiles
SCALE = 1.0 / float(np.sqrt(HD))
NEG = -1e30


def build_program(niter=1):
    nc = bacc.Bacc("TRN2", target_bir_lowering=False, debug=False,
                   enable_asserts=False, num_devices=8)
    # packed inputs: XT[p, t*2048+j] = x[j, t*128+p]
    XT = nc.dram_tensor("XT", [128, NDT * SEQ], BF16, kind="ExternalInput").ap()
    # WC[p, t*768+c] = concat(Wq_g|Wk_g|Wv_g)[t*128+p, c]
    WC = nc.dram_tensor("WC", [128, NDT * 768], BF16, kind="ExternalInput").ap()
    # WO[p, s*2048+j] = Wo[g*512+s*128+p, j]
    WO = nc.dram_tensor("WO", [128, NHL * D], BF16, kind="ExternalInput").ap()
    NM = nc.dram_tensor("NM", [128, 128], F32, kind="ExternalInput").ap()
    ONESC = nc.dram_tensor("ONESC", [128, 1], BF16, kind="ExternalInput").ap()
    ONESR = nc.dram_tensor("ONESR", [1, 128], F32R, kind="ExternalInput").ap()
    IDENT = nc.dram_tensor("IDENT", [128, 128], BF16, kind="ExternalInput").ap()
    # OUT[p, m*2048+j] = out[m*128+p, j]
    OUT = nc.dram_tensor("out", [128, NKT * D], BF16, kind="ExternalOutput").ap()

    with tile.TileContext(nc) as tc:
        with (tc.For_i(0, niter, 1) if niter > 1 else nullcontext()):
          with ExitStack() as octx:
              const = octx.enter_context(tc.tile_pool(name="const", bufs=1))
              wopool = octx.enter_context(tc.tile_pool(name="wopool", bufs=1))
              resid = octx.enter_context(tc.tile_pool(name="resid", bufs=1))

              negmask = const.tile([128, 128], F32)
              onesc = const.tile([128, 1], BF16)
              onesr = const.tile([1, 128], F32R)
              ident = const.tile([128, 128], BF16)

              # Resident SBUF arrays spanning stages (bf16).
              qT = [resid.tile([128, SEQ], BF16, name=f"qT{s}", tag=f"qT{s}")
                    for s in range(NHL)]
              kT = resid.tile([128, SEQ], BF16, name="kT", tag="kT")
              # vcat[:, ki*128:+128] = v[ki*128:(ki+1)*128, :]  ([seq-in-tile, hd])
              vcat = resid.tile([128, SEQ], BF16, name="vcat", tag="vcat")
              # ctxall[:, s*2048 + q] = ctx head s  ([hd, seq])
              ctxall = resid.tile([128, NHL * SEQ], BF16, name="ctxall",
                                  tag="ctxall")
              wo = wopool.tile([128, NHL * D], BF16, name="wo", tag="wo")

              # ---------------- Stage 1: projections ----------------
              with ExitStack() as s1:
                xpool = s1.enter_context(tc.tile_pool(name="xpool", bufs=1))
                wcpool = s1.enter_context(tc.tile_pool(name="wcpool", bufs=1))
                vtpool = s1.enter_context(tc.tile_pool(name="vtpool", bufs=2))
                pps = s1.enter_context(
                    tc.tile_pool(name="proj_ps", bufs=6, space="PSUM"))
                trps = s1.enter_context(
                    tc.tile_pool(name="tr_ps", bufs=2, space="PSUM"))

                xt = xpool.tile([128, NDT * SEQ], BF16, name="xt", tag="xt")
                wc = wcpool.tile([128, NDT * 768], BF16, name="wc", tag="wc")
                # interleave wc-part and x-group DMAs so compute starts early;
                # first groups are small so the first matmuls start ASAP
                xsplit = [0, 1, 2, 4, 7, 10, 13, 16]
                wsplit = [0, 2, 4, 8, 16]
                for gx in range(len(xsplit) - 1):
                    if gx < len(wsplit) - 1:
                        ws = slice(wsplit[gx] * 768, wsplit[gx + 1] * 768)
                        nc.sync.dma_start(wc[:, ws], WC[:, ws])
                    cs = slice(xsplit[gx] * SEQ, xsplit[gx + 1] * SEQ)
                    nc.sync.dma_start(xt[:, cs], XT[:, cs])
                nc.sync.dma_start(negmask[:], NM[:, :])
                nc.sync.dma_start(onesc[:], ONESC[:, :])
                nc.sync.dma_start(onesr[:], ONESR[:, :])
                nc.sync.dma_start(ident[:], IDENT[:, :])
                # Wo prefetch (lands during stage 2)
                nc.sync.dma_start(wo[:], WO[:, :])

                for c in range(NQC):
                    cs = slice(c * QC, (c + 1) * QC)
                    psq = [pps.tile([128, QC], F32, name=f"psq{s}_{c}",
                                    tag="proj") for s in range(NHL)]
                    psk = pps.tile([128, QC], F32, name=f"psk{c}", tag="proj")
                    psv = pps.tile([128, QC], F32, name=f"psv{c}", tag="proj")
                    for d in range(NDT):
                        xs = xt[:, d * SEQ + c * QC: d * SEQ + (c + 1) * QC]
                        st = (d == 0)
                        sp = (d == NDT - 1)
                        for s in range(NHL):
                            nc.tensor.matmul(psq[s][:],
                                             wc[:, d*768+s*HD: d*768+(s+1)*HD],
                                             xs, start=st, stop=sp)
                        nc.tensor.matmul(psk[:], wc[:, d*768+512: d*768+640],
                                         xs, start=st, stop=sp)
                        nc.tensor.matmul(psv[:], wc[:, d*768+640: d*768+768],
                                         xs, start=st, stop=sp)
                    for s in range(NHL):
                        nc.any.tensor_copy(qT[s][:, cs], psq[s][:])
                    nc.any.tensor_copy(kT[:, cs], psk[:])
                    # v: evict vT chunk (bf16), then PE-transpose 128x128 blocks
                    vt = vtpool.tile([128, QC], BF16, name=f"vt{c}", tag="vt")
                    nc.any.tensor_copy(vt[:], psv[:])
                    for t in range(QC // 128):
                        trp = trps.tile([128, 128], BF16, name=f"tr{c}_{t}",
                                        tag="tr")
                        nc.tensor.transpose(trp[:], vt[:, t*128:(t+1)*128],
                                            ident[:])
                        col = c * QC + t * 128
                        nc.any.tensor_copy(vcat[:, col:col+128], trp[:])

              # ---------------- Stage 2: attention ----------------
              with ExitStack() as s2:
                epool = s2.enter_context(tc.tile_pool(name="epool", bufs=12))
                rpool = s2.enter_context(tc.tile_pool(name="rpool", bufs=8))
                stps = s2.enter_context(
                    tc.tile_pool(name="st_ps", bufs=2, space="PSUM"))
                ctxps = s2.enter_context(
                    tc.tile_pool(name="ctx_ps", bufs=4, space="PSUM"))
                rsps = s2.enter_context(
                    tc.tile_pool(name="rs_ps", bufs=2, space="PSUM"))

                for c in range(NQC):
                    cs = slice(c * QC, (c + 1) * QC)
                    ktmax = 4 * (c + 1)
                    ctxp = [ctxps.tile([128, QC], F32, name=f"ctxp{h}_{c}",
                                       tag="ctxp") for h in range(NHL)]
                    # 2 heads' row-sums per PSUM bank (partitions 0 / 64)
                    rsb = [rsps.tile([128, QC], F32, name=f"rsb{b}_{c}",
                                     tag="rsb") for b in range(2)]
                    rs_ap = [rsb[h // 2][(h % 2) * 64:(h % 2) * 64 + 1, :]
                             for h in range(NHL)]
                    for ki in range(ktmax):
                        j = ki - 4 * c
                        n0 = 0 if j < 0 else 128 * j
                        ns = slice(n0, QC)
                        ests = []
                        for h in range(NHL):
                            stt = stps.tile([128, QC], F32,
                                            name=f"st{h}_{c}_{ki}", tag="st")
                            nc.tensor.matmul(
                                stt[:, ns], kT[:, ki*128:(ki+1)*128],
                                qT[h][:, c*QC+n0:(c+1)*QC],
                                start=True, stop=True)
                            if j >= 0:
                                nc.vector.tensor_tensor(
                                    stt[:, n0:n0+128], stt[:, n0:n0+128],
                                    negmask[:], ALU.add)
                            est = epool.tile([128, QC], BF16,
                                             name=f"est{h}_{c}_{ki}",
                                             tag="est")
                            nc.scalar.activation(est[:, ns], stt[:, ns],
                                                 AF.Exp, scale=SCALE)
                            ests.append(est)
                        for h in range(NHL):
                            nc.tensor.matmul(ctxp[h][:, ns],
                                             vcat[:, ki*128:(ki+1)*128],
                                             ests[h][:, ns],
                                             start=(ki == 0),
                                             stop=(ki == ktmax - 1))
                            nc.tensor.matmul(rs_ap[h][:, ns], onesc[:],
                                             ests[h][:, ns],
                                             start=(ki == 0),
                                             stop=(ki == ktmax - 1))
                    for h in range(NHL):
                        recip = rpool.tile([1, QC], F32R,
                                           name=f"recip{h}_{c}", tag="recip")
                        with nc.allow_low_precision(
                                reason="fp32r recip, fp32r matmul"):
                            nc.vector.reciprocal(recip[:], rs_ap[h])
                        bcp = rsps.tile([128, QC], F32, name=f"bc{h}_{c}",
                                        tag="rsb")
                        nc.tensor.matmul(bcp[:], onesr[:], recip[:],
                                         start=True, stop=True)
                        ocol = h * SEQ + c * QC
                        nc.vector.tensor_copy(
                            ctxall[:, ocol:ocol + QC], ctxp[h][:])
                        nc.vector.tensor_tensor(
                            ctxall[:, ocol:ocol + QC],
                            ctxall[:, ocol:ocol + QC], bcp[:], ALU.mult)

              # ---------------- Stage 3: output projection ----------------
              with ExitStack() as s3:
                opool = s3.enter_context(tc.tile_pool(name="opool", bufs=4))
                ops = s3.enter_context(
                    tc.tile_pool(name="out_ps", bufs=8, space="PSUM"))
                for m in range(SEQ // 128):
                    pso = [ops.tile([128, 512], F32, name=f"pso{m}_{n}",
                                    tag="pso") for n in range(4)]
                    for s in range(NHL):
                        lhs = ctxall[:, s*SEQ+m*128: s*SEQ+(m+1)*128]
                        for n in range(4):
                            nc.tensor.matmul(pso[n][:], lhs,
                                             wo[:, s*D+n*512:(s*D+(n+1)*512)],
                                             start=(s == 0),
                                             stop=(s == NHL - 1))
                    ot = opool.tile([128, D], BF16, name=f"ot{m}", tag="ot")
                    for n in range(4):
                        nc.any.tensor_copy(ot[:, n*512:(n+1)*512], pso[n][:])
                    nc.sync.dma_start(OUT[:, m*D:(m+1)*D], ot[:])

    nc.compile()
    nc.m = get_hw_module(nc.m)
    return nc


_NC = None


def _get_nc():
    global _NC
    if _NC is None:
        _NC = build_program()
    return _NC


def _consts():
    negmask = np.where(np.arange(128)[:, None] <= np.arange(128)[None, :],
                       0.0, NEG).astype(np.float32)
    return {
        "NM": negmask,
        "ONESC": np.ones((128, 1), BF),
        "ONESR": np.ones((1, 128), np.float32),
        "IDENT": np.eye(128, dtype=BF),
    }


def _pack(a, ntile):
    """[ntile*128, N] f32 -> [128, ntile*N] bf16 with tile t at cols t*N."""
    n = a.shape[1]
    return np.ascontiguousarray(
        a.reshape(ntile, 128, n).transpose(1, 0, 2).reshape(128, ntile * n)
    ).astype(BF)


def make_in_maps(x, Wq, Wk, Wv, Wo):
    consts = _consts()
    in_maps = []
    xTp = [None, None]
    for i in range(8):
        bi, g = i // 4, i % 4
        if xTp[bi] is None:
            xTp[bi] = _pack(np.ascontiguousarray(x[bi].T), NDT)
        wcat = np.concatenate([Wq[:, g*512:(g+1)*512],
                               Wk[:, g*128:(g+1)*128],
                               Wv[:, g*128:(g+1)*128]], axis=1)
        in_maps.append({
            "XT": xTp[bi],
            "WC": _pack(wcat, NDT),
            "WO": _pack(np.ascontiguousarray(Wo[g*512:(g+1)*512, :]), NHL),
            **consts,
        })
    return in_maps


def _unpack_out(o):
    """[128, 16*2048] bf16 -> [2048, 2048] f32."""
    return o.reshape(128, NKT, D).transpose(1, 0, 2).reshape(SEQ, D).astype(
        np.float32)


def kernel(x, Wq, Wk, Wv, Wo, bo):
    x = np.asarray(x, np.float32)
    Wq = np.asarray(Wq, np.float32)
    Wk = np.asarray(Wk, np.float32)
    Wv = np.asarray(Wv, np.float32)
    Wo = np.asarray(Wo, np.float32)
    bo = np.asarray(bo, np.float32)
    b = x.shape[0]
    nc = _get_nc()
    in_maps = make_in_maps(x, Wq, Wk, Wv, Wo)
    res = bass_utils.run_bass_kernel_spmd(nc, in_maps,
                                          core_ids=list(range(8)),
                                          trace=False)
    out = np.zeros((b, SEQ, D), np.float32)
    for i in range(8):
        bi = i // 4
        out[bi] += _unpack_out(np.asarray(res.results[i]["out"]))
    out += bo[None, None, :]
    return out
